# revision 11
# baseline (speedup 1.0000x reference)
"""CoAttention kernel for one TRN2 chip (8 NeuronCores, batch-parallel).

Reference computation (per batch b):
    Q   = x1 @ Wq^T                      [N1, D]
    L   = Q @ x2^T                       [N1, N2]
    A_D = softmax(L, axis=2)             [N1, N2]   (row softmax)
    C_D = A_D^T @ Q                      [N2, D]
    A_Q = softmax(L, axis=1)^T           [N2, N1]   (= row softmax of L^T)
    C_Q = A_Q^T @ x2                     [N1, D]
returns (C_Q, C_D, A_D, A_Q).

Strategy: data-parallel over batch (16 batches / 8 cores = 2 per core).
Host pre-transposes x1, x2, Wq so the device only ever does natural-layout
DMAs.  On the device, per batch:
  - project Q (natural [N1,D]) and Q^T ([D,N1]) from x1^T and Wq^T
  - L row-blocks of 128 via matmul(lhsT=Q^T chunk, rhs=x2^T chunk),
    flash-style chunked softmax (chunk-local max + global rescale) -> A_D
    streamed to HBM
  - L^T row-blocks symmetrically -> A_Q streamed to HBM
  - C_D accumulated over n-blocks in PSUM from A_D tiles re-read from HBM
    (lhsT=A_D tile, rhs=Q natural); C_Q symmetrically from A_Q and x2.
"""

import os
import numpy as np
from contextlib import ExitStack

import concourse.bass as bass
from concourse import bacc
import concourse.mybir as mybir
import concourse.tile as tile
from concourse.bass_utils import run_bass_kernel_spmd

FP32 = mybir.dt.float32
P = 128
F = 512  # matmul moving free dim / psum bank width (fp32)

# full-problem constants (hardcoded per harness contract)
B_FULL = 16
N_CORES = 8
BPC = B_FULL // N_CORES  # batches per core
N1_FULL = 2048
N2_FULL = 2048
D_FULL = 512


def build_nc(BPC, N1, N2, D, mm_dtype=mybir.dt.float32r):
    """Build the single-core program (SPMD-replicated across cores)."""
    DC = D // P    # contraction chunks over the model dim
    NB1 = N1 // P  # 128-row blocks of L
    NB2 = N2 // P  # 128-row blocks of L^T
    N1F = N1 // F  # 512-wide chunks of N1
    N2F = N2 // F  # 512-wide chunks of N2
    assert D % F == 0 and N1 % F == 0 and N2 % F == 0

    nc = bacc.Bacc(target_bir_lowering=False)

    MMDT = mm_dtype
    x1t = nc.declare_dram_parameter("x1t", [BPC, D, N1], MMDT, isOutput=False)
    x2d = nc.declare_dram_parameter("x2", [BPC, N2, D], MMDT, isOutput=False)
    x2t = nc.declare_dram_parameter("x2t", [BPC, D, N2], MMDT, isOutput=False)
    wqt = nc.declare_dram_parameter("wqt", [D, D], MMDT, isOutput=False)
    cq = nc.declare_dram_parameter("cq", [BPC, N1, D], FP32, isOutput=True)
    cd = nc.declare_dram_parameter("cd", [BPC, N2, D], FP32, isOutput=True)
    ad = nc.declare_dram_parameter("ad", [BPC, N1, N2], FP32, isOutput=True)
    aq = nc.declare_dram_parameter("aq", [BPC, N2, N1], FP32, isOutput=True)

    EXP = mybir.ActivationFunctionType.Exp
    AXX = mybir.AxisListType.X

    def mm(ps, lhsT, rhs, start, stop):
        nc.tensor.matmul(ps, lhsT, rhs, start=start, stop=stop)


    with tile.TileContext(nc) as tc, ExitStack() as ctx:
        pw = ctx.enter_context(tc.tile_pool(name="pwqt", bufs=1))
        px1 = ctx.enter_context(tc.tile_pool(name="px1", bufs=2))
        pq = ctx.enter_context(tc.tile_pool(name="pq", bufs=1))
        pqt = ctx.enter_context(tc.tile_pool(name="pqt", bufs=1))
        px2 = ctx.enter_context(tc.tile_pool(name="px2", bufs=1))
        px2t = ctx.enter_context(tc.tile_pool(name="px2t", bufs=1))
        pa = ctx.enter_context(tc.tile_pool(name="pa", bufs=3))
        pst = ctx.enter_context(tc.tile_pool(name="pst", bufs=4))
        pcin = ctx.enter_context(tc.tile_pool(name="pcin", bufs=4))
        pcout = ctx.enter_context(tc.tile_pool(name="pcout", bufs=3))
        lps = ctx.enter_context(tc.tile_pool(name="lps", bufs=4, space="PSUM"))
        cps = ctx.enter_context(tc.tile_pool(name="cps", bufs=4, space="PSUM"))

        wqt_sb = pw.tile([P, DC, D], MMDT, tag="wqt")
        nc.sync.dma_start(wqt_sb[:], wqt.rearrange("(c p) e -> p c e", p=P))

        def softmax_rows(lhs_sb, rhs_sb, nblocks, nfree_chunks, out_dram_b):
            """lhs_sb: [P, DC, nblocks*P] (transposed operand whose columns
            become psum partitions); rhs_sb: [P, DC, nfree_chunks*F];
            writes row-softmaxed blocks to out_dram_b [nblocks*P, nfree*F]."""
            for nb in range(nblocks):
                mx = pst.tile([P, nfree_chunks], FP32, tag="mx")
                rs = pst.tile([P, nfree_chunks], FP32, tag="rs")
                p_sb = pa.tile([P, nfree_chunks * F], FP32, tag="arow")
                for mf in range(nfree_chunks):
                    ps = lps.tile([P, F], FP32, tag="lps")
                    for dc in range(DC):
                        mm(
                            ps,
                            lhs_sb[:, dc, nb * P : (nb + 1) * P],
                            rhs_sb[:, dc, mf * F : (mf + 1) * F],
                            dc == 0,
                            dc == DC - 1,
                        )
                    # chunk-local max (negated -> bias for exp)
                    nmx = pst.tile([P, 1], FP32, tag="nmx")
                    nc.vector.reduce_max(nmx, ps, axis=AXX, negate=True)
                    nc.vector.tensor_scalar_mul(mx[:, mf : mf + 1], nmx, -1.0)
                    # exp(chunk - chunkmax), accumulate row sums
                    nc.scalar.activation(
                        p_sb[:, mf * F : (mf + 1) * F],
                        ps,
                        EXP,
                        bias=nmx,
                        accum_out=rs[:, mf : mf + 1],
                    )
                # global max + per-chunk rescale: A = P_mf * t_mf / Z,
                # t_mf = exp(mx_mf - gmax), Z = sum_mf rs_mf * t_mf
                ngmx = pst.tile([P, 1], FP32, tag="ngmx")
                nc.vector.reduce_max(ngmx, mx, axis=AXX, negate=True)
                t = pst.tile([P, nfree_chunks], FP32, tag="t")
                nc.scalar.activation(t, mx, EXP, bias=ngmx)
                zt = pst.tile([P, nfree_chunks], FP32, tag="zt")
                nc.vector.tensor_mul(zt, rs, t)
                z = pst.tile([P, 1], FP32, tag="z")
                nc.vector.reduce_sum(z, zt, axis=AXX)
                zinv = pst.tile([P, 1], FP32, tag="zinv")
                nc.vector.reciprocal(zinv, z)
                sc = pst.tile([P, nfree_chunks], FP32, tag="sc")
                nc.vector.tensor_scalar_mul(sc, t, zinv)
                for mf in range(nfree_chunks):
                    nc.vector.tensor_scalar_mul(
                        p_sb[:, mf * F : (mf + 1) * F],
                        p_sb[:, mf * F : (mf + 1) * F],
                        sc[:, mf : mf + 1],
                    )
                nc.sync.dma_start(out_dram_b[nb * P : (nb + 1) * P, :], p_sb)

        def context_panel(pf, a_dram_b, rhs_sb, nblocks, out_dram_b):
            """C panel: out rows pf*F..pf*F+F (4 tiles of 128), all D cols.
            a_dram_b: [nblocks*P, *]; rhs_sb: [P, nblocks, D]."""
            ctiles = [cps.tile([P, F], FP32, tag="cps", name=f"cps{_i}") for _i in range(F // P)]
            for nb in range(nblocks):
                at = pcin.tile([P, F], MMDT, tag="cin")
                nc.sync.dma_start(
                    at,
                    a_dram_b[nb * P : (nb + 1) * P, pf * F : (pf + 1) * F].bitcast(
                        MMDT
                    ),
                )
                for mt in range(F // P):
                    mm(
                        ctiles[mt],
                        at[:, mt * P : (mt + 1) * P],
                        rhs_sb[:, nb, :],
                        nb == 0,
                        nb == nblocks - 1,
                    )
            for mt in range(F // P):
                co = pcout.tile([P, F], FP32, tag="cout")
                nc.scalar.copy(co, ctiles[mt])
                nc.sync.dma_start(
                    out_dram_b[pf * F + mt * P : pf * F + (mt + 1) * P, :], co
                )

        for b in range(BPC):
            # ---------------- projection: Q natural + Q^T ----------------
            q_sb = pq.tile([P, NB1, D], MMDT, tag="q")
            qt_sb = pqt.tile([P, DC, N1], MMDT, tag="qt")
            for nf in range(N1F):
                x1c = px1.tile([P, DC, F], MMDT, tag="x1c")
                nc.sync.dma_start(
                    x1c,
                    x1t[b].rearrange("(c p) n -> p c n", p=P)[
                        :, :, nf * F : (nf + 1) * F
                    ],
                )
                for j in range(F // P):
                    nb = nf * (F // P) + j
                    ps = lps.tile([P, D], FP32, tag="lps")
                    for dc in range(DC):
                        mm(
                            ps,
                            x1c[:, dc, j * P : (j + 1) * P],
                            wqt_sb[:, dc, :],
                            dc == 0,
                            dc == DC - 1,
                        )
                    nc.scalar.copy(q_sb[:, nb, :], ps)
                for eb in range(DC):
                    ps = lps.tile([P, F], FP32, tag="lps")
                    for dc in range(DC):
                        mm(
                            ps,
                            wqt_sb[:, dc, eb * P : (eb + 1) * P],
                            x1c[:, dc, :],
                            dc == 0,
                            dc == DC - 1,
                        )
                    nc.scalar.copy(qt_sb[:, eb, nf * F : (nf + 1) * F], ps)

            x2_sb = px2.tile([P, NB2, D], MMDT, tag="x2")
            nc.sync.dma_start(x2_sb, x2d[b].rearrange("(nb p) d -> p nb d", p=P))
            x2t_sb = px2t.tile([P, DC, N2], MMDT, tag="x2t")
            nc.sync.dma_start(x2t_sb, x2t[b].rearrange("(c p) m -> p c m", p=P))

            # ---------------- A_D = rowsoftmax(L) ----------------
            softmax_rows(qt_sb, x2t_sb, NB1, N2F, ad[b])
            # ---------------- A_Q = rowsoftmax(L^T), interleave C_D -------
            # (C_D needs all of A_D, which is done; interleave its panels
            # with the L^T blocks so DMA/PE stay dense.)
            blocks_per_panel = max(1, NB2 // max(1, N2F))
            for mb in range(NB2):
                # one L^T row block -> one 128-row block of A_Q
                softmax_rows(
                    x2t_sb[:, :, mb * P : (mb + 1) * P],
                    qt_sb,
                    1,
                    N1F,
                    aq[b][mb * P : (mb + 1) * P, :],
                )
                if (mb + 1) % blocks_per_panel == 0:
                    pf = (mb + 1) // blocks_per_panel - 1
                    if pf < N2F:
                        context_panel(pf, ad[b], q_sb, NB1, cd[b])
            # ---------------- C_Q ----------------
            for pf in range(N1F):
                context_panel(pf, aq[b], x2_sb, NB2, cq[b])

    nc.compile()
    return nc


_NC_CACHE = {}
LAST_RESULTS = None


def _get_nc():
    key = (BPC, N1_FULL, N2_FULL, D_FULL, os.environ.get("COATT_MM_DT", "float32r"))
    if key not in _NC_CACHE:
        mm_dt = getattr(mybir.dt, key[-1])
        _NC_CACHE[key] = build_nc(BPC, N1_FULL, N2_FULL, D_FULL, mm_dt)
    return _NC_CACHE[key]


def kernel(x1, x2, node_mask=None, Wq=None, **_unused):
    x1 = np.ascontiguousarray(np.asarray(x1, dtype=np.float32))
    x2 = np.ascontiguousarray(np.asarray(x2, dtype=np.float32))
    Wq = np.ascontiguousarray(np.asarray(Wq, dtype=np.float32))
    B = x1.shape[0]
    assert B == B_FULL and x1.shape == (B, N1_FULL, D_FULL)

    x1t = np.ascontiguousarray(x1.transpose(0, 2, 1))
    x2t = np.ascontiguousarray(x2.transpose(0, 2, 1))
    wqt = np.ascontiguousarray(Wq.T)

    nc = _get_nc()
    in_maps = []
    for c in range(N_CORES):
        s = slice(c * BPC, (c + 1) * BPC)
        in_maps.append(
            {"x1t": x1t[s], "x2": x2[s], "x2t": x2t[s], "wqt": wqt}
        )
    trace = bool(int(os.environ.get("COATT_TRACE", "0")))
    res = run_bass_kernel_spmd(nc, in_maps, list(range(N_CORES)), trace=trace)
    global LAST_RESULTS
    LAST_RESULTS = res
    outs = res.results
    cq = np.concatenate([outs[c]["cq"] for c in range(N_CORES)], axis=0)
    cd = np.concatenate([outs[c]["cd"] for c in range(N_CORES)], axis=0)
    ad = np.concatenate([outs[c]["ad"] for c in range(N_CORES)], axis=0)
    aq = np.concatenate([outs[c]["aq"] for c in range(N_CORES)], axis=0)
    return (cq, cd, ad, aq)
